# revision 15
# baseline (speedup 1.0000x reference)
"""Bilateral slice kernel for Trainium2 (8 NeuronCores, SPMD).

Problem (hardcoded shapes):
  grid  [B=4, C=12, Dg=8, Hg=16, Wg=16] f32
  guide [B=4, 1, H=1024, W=1024] f32
  out   [B=4, C=12, H=1024, W=1024] f32

Sharding: pure data parallel. Core i handles batch b = i//2, row half
r0 = (i%2)*512. No cross-core communication.

Algorithm per core, per 32-row group, depth planes split into lo
(d=0..3) / hi (d=4..7) stacks of (d4,y32)=128 partitions:
  S[c] = gT[c].T @ Ax           x-interp once per channel (PE, f32r->bf16)
  P_lo/P_hi = Ay_blk.T @ S[c]   y-interp (PE, bf16, 2x512-col per stack)
  w8 = relu(1-|7*guide - d|)    z-hat weights (ACT, bf16; guide rows
                                DMA-replicated to 128 partitions)
  v = w8 * P                    lo: DVE direct from PSUM (1x);
                                hi: ACT copies PSUM->SBUF bf16 for 5/6 of
                                channels, then all-bf16 mult (2x_1p) on
                                DVE (GPSIMD for c in {4,10})
  out32 = sel32.T @ [v_lo;v_hi] depth-reduce, lo/hi merged for free by
                                PSUM accumulation; tile_position=(0,32*cc)
                                packs 4 channels into one PSUM tile so one
                                ACT copy stages 4 channels (bf16 out)

Stage A is software-pipelined into the g=0 channel loop (2-channel
lookahead, sa tiles borrow ps_hi slots); ay/sel constants are fed from
the host directly in bf16. Engine balance: PE ~397us busy (pacer),
DVE ~350us work, ACT ~310us. Output is bf16 (rel err 7.4e-3 vs 2e-2
tolerance). hi-stack matmuls are emitted before lo so the ACT egress
copy gets a head start on the DVE multiply that consumes it. Measured
428.4-430.8us on 8 cores (baseline 585.9us); device p-state throttling
occasionally inflates a run ~20%.

Notes from tuning (see transcript): scalar_tensor_tensor never engages
DVE 2x on HW (stays 1x) — plain tensor_tensor all-bf16 packed does;
GPSIMD tensor_tensor costs ~2.1us/[128,1024] (eff 0.42) and putting it
on the critical path drops engine p-states; stall-heavy schedules
inflate busy times ~20% (p-state), so the shallow PE->DVE->PE graph
with PSUM-accumulated merge beats deeper "balanced" graphs.
"""

import sys
import numpy as np

for _p in ("/opt/trn_rl_repo",):
    if _p not in sys.path:
        sys.path.insert(0, _p)

B, C, Dg, Hg, Wg = 4, 12, 8, 16, 16
H, W = 1024, 1024
N_CORES = 8
ROWS_PER_CORE = H // 2          # 512
N_G32 = ROWS_PER_CORE // 32     # 16 groups of 32 rows
NQ = C // 4                     # 3 channel quads


def _hat_matrix(n_out: int, n_lat: int) -> np.ndarray:
    i = np.linspace(0.0, n_lat - 1.0, n_out, dtype=np.float32)
    lat = np.arange(n_lat, dtype=np.float32)[:, None]
    return np.maximum(0.0, 1.0 - np.abs(i[None, :] - lat)).astype(np.float32)


def _build_tables():
    ax = _hat_matrix(W, Wg)                      # [16, 1024]
    ay = _hat_matrix(H, Hg)                      # [16, 1024]
    # sel32[(dd,y), y'] = -(y==y') : reduces 4 depth blocks -> 32 rows.
    # NEGATED because v tiles carry -w8*P (w8_neg = min(u-1,0) = -relu(1-u)
    # is one fused DVE tensor_scalar op); the sign cancels in the matmul.
    sel32 = np.zeros((128, 32), np.float32)
    for dd in range(4):
        sel32[dd * 32:(dd + 1) * 32, :] = -np.eye(32, dtype=np.float32)
    bias_lo = np.repeat(-np.arange(0, 4, dtype=np.float32), 32)[:, None]
    bias_hi = np.repeat(-np.arange(4, 8, dtype=np.float32), 32)[:, None]

    ay_bd = {}
    for r0 in (0, ROWS_PER_CORE):
        both = []
        for off in (0, 4):
            blk = np.zeros((N_G32, 128, 128), np.float32)
            a = ay[:, r0:r0 + ROWS_PER_CORE].reshape(16, N_G32, 32).transpose(1, 0, 2)
            for dd in range(4):
                d = dd + off
                blk[:, d * 16:(d + 1) * 16, dd * 32:(dd + 1) * 32] = a
            both.append(blk.transpose(1, 0, 2).reshape(128, N_G32 * 128))
        ay_bd[r0] = np.ascontiguousarray(
            np.stack(both, 1).reshape(128, 2 * N_G32 * 128))
    return dict(ax=ax, sel32=sel32, bias_lo=bias_lo, bias_hi=bias_hi, ay_bd=ay_bd)


def _build_nc():
    from contextlib import ExitStack
    import concourse.bass as bass
    import concourse.bacc as bacc
    import concourse.tile as tile
    import concourse.mybir as mybir

    f32 = mybir.dt.float32
    f32r = mybir.dt.float32r
    bf16 = mybir.dt.bfloat16
    AF = mybir.ActivationFunctionType
    Alu = mybir.AluOpType

    nc = bacc.Bacc("TRN2", target_bir_lowering=False, debug=False)

    guide_d = nc.dram_tensor("guide", [ROWS_PER_CORE, W], f32, kind="ExternalInput")
    gT_d = nc.dram_tensor("gT", [16, C * 128], f32, kind="ExternalInput")
    ax_d = nc.dram_tensor("ax", [16, W], f32, kind="ExternalInput")
    aybd_d = nc.dram_tensor("aybd", [128, 2 * N_G32 * 128], bf16, kind="ExternalInput")
    sel_d = nc.dram_tensor("sel32", [128, 32], bf16, kind="ExternalInput")
    bias_lo_d = nc.dram_tensor("bias_lo", [128, 1], f32, kind="ExternalInput")
    bias_hi_d = nc.dram_tensor("bias_hi", [128, 1], f32, kind="ExternalInput")
    # out[q, g, (cc,y32), (h,x)] bf16
    out_d = nc.dram_tensor("out", [NQ, N_G32, 128, 2 * 512], bf16,
                           kind="ExternalOutput")

    with tile.TileContext(nc) as tc, ExitStack() as ctx:
        const = ctx.enter_context(tc.tile_pool(name="const", bufs=1))
        ax_t = const.tile([16, W], f32)
        nc.sync.dma_start(ax_t[:], ax_d[:])
        gT_t = const.tile([16, C * 128], f32)
        nc.sync.dma_start(gT_t[:], gT_d[:])
        ay_r = const.tile([128, 2 * N_G32 * 128], bf16)
        nc.sync.dma_start(ay_r[:], aybd_d[:])
        sel_b = const.tile([128, 32], bf16)
        nc.sync.dma_start(sel_b[:], sel_d[:])
        bias_lo_t = const.tile([128, 1], f32)
        nc.sync.dma_start(bias_lo_t[:], bias_lo_d[:])
        bias_hi_t = const.tile([128, 1], f32)
        nc.sync.dma_start(bias_hi_t[:], bias_hi_d[:])
        # PE warmup: dummy matmuls on a zeroed tile so the HAM activity
        # window sees a busy PE and ramps 1.2->2.4 GHz before real work.
        # Emitted FIRST so the DVE queue runs them before the const copies
        # (which block on their DMAs).
        wm_t = const.tile([16, 512], f32)
        nc.vector.memset(wm_t[:], 0.0)
        wm_r = const.tile([16, 512], f32r)
        nc.vector.tensor_copy(wm_r[:], wm_t[:])

        gT_r = const.tile([16, C * 128], f32r)
        nc.vector.tensor_copy(gT_r[:], gT_t[:])
        ax_r = const.tile([16, W], f32r)
        nc.vector.tensor_copy(ax_r[:], ax_t[:])

        s_pool = ctx.enter_context(tc.tile_pool(name="s_all", bufs=1))
        s_tiles = []
        for c in range(C):
            s_c = s_pool.tile([128, W], bf16, tag=f"s{c}", name=f"s{c}")
            s_tiles.append(s_c)

        iz_pool = ctx.enter_context(tc.tile_pool(name="iz128", bufs=3))
        u8_pool = ctx.enter_context(tc.tile_pool(name="u8", bufs=4))
        w8_pool = ctx.enter_context(tc.tile_pool(name="w8", bufs=4))
        phs_pool = ctx.enter_context(tc.tile_pool(name="phs", bufs=6))
        vlo_pool = ctx.enter_context(tc.tile_pool(name="vlo", bufs=6))
        vhi_pool = ctx.enter_context(tc.tile_pool(name="vhi", bufs=6))
        # unified p8 PSUM pool: 3 x [128,1024] f32 = 6 banks; each (g,c)
        # draws hi then lo, so the pipeline is ~1.5 (g,c) deep uniformly.
        ps_p8 = ctx.enter_context(tc.tile_pool(name="ps_p8", bufs=3, space="PSUM"))
        ps_out = ctx.enter_context(tc.tile_pool(name="ps_out", bufs=1, space="PSUM"))
        ob_pool = ctx.enter_context(tc.tile_pool(name="ob", bufs=6))

        # Stage A (x-interp S[c] = gT[c].T @ Ax) is pipelined into the
        # g=0 channel loop with 2-channel lookahead; sa tiles borrow the
        # ps_p8 pool slots to stay within the 8 PSUM banks.
        def emit_stA(c):
            sa = ps_p8.tile([128, W], f32, tag="p8")
            for h in range(2):
                nc.tensor.matmul(
                    sa[:, h * 512:(h + 1) * 512],
                    gT_r[:, c * 128:(c + 1) * 128],
                    ax_r[:, h * 512:(h + 1) * 512],
                    start=True, stop=True)
            nc.scalar.copy(s_tiles[c][:], sa[:])

        for _ in range(6):
            wmp = ps_p8.tile([128, W], f32, tag="p8")
            for h in range(2):
                nc.tensor.matmul(wmp[:, h * 512:(h + 1) * 512],
                                 wm_r[:, :128], wm_r[:],
                                 start=True, stop=True)

        for c in range(2):
            emit_stA(c)

        # software-pipelined red: holds (v, c) pending (2-deep so slow
        # GPSIMD multiplies never stall the PE reduce matmuls)
        pending = []

        def emit_red(force=False):
            while pending and (force or len(pending) > 2):
                vs, c, otiles = pending.pop(0)
                cc = c % 4
                # h=1 first: its staging copy is on DVE (faster drain), so
                # the next quad's o1-writing matmuls unblock sooner.
                for h in (1, 0):
                    for i, v in enumerate(vs):
                        nc.tensor.matmul(
                            otiles[h][cc * 32:(cc + 1) * 32, :],
                            sel_b[:],
                            v[:, h * 512:(h + 1) * 512],
                            start=(i == 0), stop=(i == len(vs) - 1),
                            tile_position=(0, cc * 32),
                        )
                if cc == 3:
                    g = otiles[2]
                    q = c // 4
                    # one [128,1024] staging tile, halves filled by ACT and
                    # DVE in parallel, shipped in a single DMA
                    ob = ob_pool.tile([128, 2 * 512], bf16, tag="ob")
                    nc.vector.tensor_copy(ob[:, 512:], otiles[1][:])
                    nc.scalar.copy(ob[:, :512], otiles[0][:])
                    nc.sync.dma_start(out_d[q, g, :, :], ob[:])

        # w8(g) is computed on ACT one group AHEAD (during g-1's slack
        # window at the copy-free direct channel) so DVE never stalls on
        # the abs/relu chain at a group boundary; guide DMA prefetched.
        def load_iz(g):
            iz = iz_pool.tile([128, W], f32)
            for r in range(4):
                nc.sync.dma_start(iz[r * 32:(r + 1) * 32, :],
                                  guide_d[bass.ts(g, 32), :])
            return iz

        def w8_act(iz):
            # u = |7*guide - d| on ACT; w8_neg = min(u-1, 0) = -relu(1-u)
            # as ONE DVE tensor_scalar (bf16 SBUF -> 4x mode, ~330ns).
            u_lo = u8_pool.tile([128, W], bf16, tag="u8_0")
            u_hi = u8_pool.tile([128, W], bf16, tag="u8_1")
            w8_lo = w8_pool.tile([128, W], bf16, tag="w8_0")
            w8_hi = w8_pool.tile([128, W], bf16, tag="w8_1")
            nc.scalar.activation(u_lo[:], iz[:], AF.Abs,
                                 bias=bias_lo_t[:], scale=7.0)
            nc.scalar.activation(u_hi[:], iz[:], AF.Abs,
                                 bias=bias_hi_t[:], scale=7.0)
            nc.vector.tensor_scalar(w8_lo[:], u_lo[:], 1.0, 0.0,
                                    op0=Alu.subtract, op1=Alu.min)
            nc.vector.tensor_scalar(w8_hi[:], u_hi[:], 1.0, 0.0,
                                    op0=Alu.subtract, op1=Alu.min)
            return w8_lo, w8_hi

        w8_cur = w8_act(load_iz(0))
        iz_next = None
        for g in range(N_G32):
            w8_lo, w8_hi = w8_cur
            if g + 1 < N_G32:
                iz_next = load_iz(g + 1)

            otiles = None
            for c in range(C):
                if g == 0 and c + 2 < C:
                    emit_stA(c + 2)
                if c == 5 and g + 1 < N_G32:
                    w8_cur = w8_act(iz_next)
                cc = c % 4
                if cc == 0:
                    o0 = ps_out.tile([128, 512], f32, tag="o0")
                    o1 = ps_out.tile([128, 512], f32, tag="o1")
                    otiles = (o0, o1, g)
                # mm2: y-interp, both stacks
                p8_hi = ps_p8.tile([128, W], f32, tag="p8")
                p8_lo = ps_p8.tile([128, W], f32, tag="p8")
                for lh, p8 in ((1, p8_hi), (0, p8_lo)):
                    lhs_off = (lh * N_G32 + g) * 128
                    for h in range(2):
                        nc.tensor.matmul(
                            p8[:, h * 512:(h + 1) * 512],
                            ay_r[:, lhs_off:lhs_off + 128],
                            s_tiles[c][:, h * 512:(h + 1) * 512],
                            start=True, stop=True)
                # three mult paths per tile, rotated by channel so DVE /
                # ACT / GPSIMD all stay below the PE pacer:
                #   D: DVE tensor_tensor straight from PSUM (1x, ~1.19us)
                #   X: ACT copy PSUM->SBUF bf16 + DVE all-bf16 TT (2x)
                #   G: ACT copy + GPSIMD TT (~2.4us, off critical path)
                lo_path = ("X" if c in (0, 3, 6, 9) else
                           "G" if c in (2, 8) else "D")
                hi_path = ("G" if c in (0, 3, 6, 10) else
                           "X" if c in (1, 5, 8) else "D")
                # ACT copies first (egress head start), then DVE directs,
                # then the dependent bf16 multiplies.
                srcs = {}
                for path, p8 in ((hi_path, p8_hi), (lo_path, p8_lo)):
                    if path != "D":
                        ph_s = phs_pool.tile([128, W], bf16)
                        nc.scalar.copy(ph_s[:], p8[:])
                        srcs[id(p8)] = ph_s
                v_lo = vlo_pool.tile([128, W], bf16)
                v_hi = vhi_pool.tile([128, W], bf16)
                for path, p8, v, w8 in ((lo_path, p8_lo, v_lo, w8_lo),
                                        (hi_path, p8_hi, v_hi, w8_hi)):
                    if path == "D":
                        nc.vector.tensor_mul(v[:], w8[:], p8[:])
                for path, p8, v, w8 in ((hi_path, p8_hi, v_hi, w8_hi),
                                        (lo_path, p8_lo, v_lo, w8_lo)):
                    if path == "X":
                        nc.vector.tensor_tensor(v[:], srcs[id(p8)][:],
                                                w8[:], op=Alu.mult)
                    elif path == "G":
                        nc.gpsimd.tensor_tensor(v[:], srcs[id(p8)][:],
                                                w8[:], op=Alu.mult)
                pending.append(((v_lo, v_hi), c, otiles))
                emit_red()
        emit_red(force=True)

    nc.compile()
    return nc


_NC = None


def _get_nc():
    global _NC
    if _NC is None:
        _NC = _build_nc()
    return _NC


def make_in_maps(grid: np.ndarray, guide: np.ndarray):
    tabs = _build_tables()
    in_maps = []
    for core in range(N_CORES):
        b, half = core // 2, core % 2
        r0 = half * ROWS_PER_CORE
        gT = np.ascontiguousarray(
            grid[b].transpose(3, 0, 1, 2).reshape(16, C * 128))
        import ml_dtypes
        bf = ml_dtypes.bfloat16
        in_maps.append({
            "guide": np.ascontiguousarray(guide[b, 0, r0:r0 + ROWS_PER_CORE, :]),
            "gT": gT,
            "ax": tabs["ax"],
            "aybd": tabs["ay_bd"][r0].astype(bf),
            "sel32": tabs["sel32"].astype(bf),
            "bias_lo": tabs["bias_lo"],
            "bias_hi": tabs["bias_hi"],
        })
    return in_maps


def assemble(results) -> np.ndarray:
    out = np.empty((B, C, H, W), np.float32)
    for core in range(N_CORES):
        b, half = core // 2, core % 2
        r0 = half * ROWS_PER_CORE
        arr = np.asarray(results[core]["out"], dtype=np.float32)
        # arr [q, g, (cc,y32), (h,x)] -> [c, rows, W]
        arr = arr.reshape(NQ, N_G32, 4, 32, W)
        arr = arr.transpose(0, 2, 1, 3, 4).reshape(C, ROWS_PER_CORE, W)
        out[b, :, r0:r0 + ROWS_PER_CORE, :] = arr
    return out


def kernel(grid, guide, output_size):
    from concourse.bass_utils import run_bass_kernel_spmd

    grid = np.asarray(grid, dtype=np.float32)
    guide = np.asarray(guide, dtype=np.float32)
    assert grid.shape == (B, C, Dg, Hg, Wg), grid.shape
    assert guide.shape == (B, 1, H, W), guide.shape

    nc = _get_nc()
    in_maps = make_in_maps(grid, guide)
    res = run_bass_kernel_spmd(nc, in_maps, list(range(N_CORES)))
    return assemble(res.results)



# revision 16
# speedup vs baseline: 1.0004x; 1.0004x over previous
"""Bilateral slice kernel for Trainium2 (8 NeuronCores, SPMD).

Problem (hardcoded shapes):
  grid  [B=4, C=12, Dg=8, Hg=16, Wg=16] f32
  guide [B=4, 1, H=1024, W=1024] f32
  out   [B=4, C=12, H=1024, W=1024] f32

Sharding: pure data parallel. Core i handles batch b = i//2, row half
r0 = (i%2)*512. No cross-core communication.

Algorithm per core, per 32-row group, depth planes split into lo
(d=0..3) / hi (d=4..7) stacks of (d4,y32)=128 partitions:
  S[c] = gT[c].T @ Ax           x-interp once per channel (PE, f32r->bf16)
  P_lo/P_hi = Ay_blk.T @ S[c]   y-interp (PE, bf16, 2x512-col per stack)
  w8 = relu(1-|7*guide - d|)    z-hat weights (ACT, bf16; guide rows
                                DMA-replicated to 128 partitions)
  v = w8 * P                    lo: DVE direct from PSUM (1x);
                                hi: ACT copies PSUM->SBUF bf16 for 5/6 of
                                channels, then all-bf16 mult (2x_1p) on
                                DVE (GPSIMD for c in {4,10})
  out32 = sel32.T @ [v_lo;v_hi] depth-reduce, lo/hi merged for free by
                                PSUM accumulation; tile_position=(0,32*cc)
                                packs 4 channels into one PSUM tile so one
                                ACT copy stages 4 channels (bf16 out)

Stage A is software-pipelined into the g=0 channel loop (2-channel
lookahead, sa tiles borrow ps_hi slots); ay/sel constants are fed from
the host directly in bf16. Engine balance: PE ~397us busy (pacer),
DVE ~350us work, ACT ~310us. Output is bf16 (rel err 7.4e-3 vs 2e-2
tolerance). hi-stack matmuls are emitted before lo so the ACT egress
copy gets a head start on the DVE multiply that consumes it. Measured
428.4-430.8us on 8 cores (baseline 585.9us); device p-state throttling
occasionally inflates a run ~20%.

Notes from tuning (see transcript): scalar_tensor_tensor never engages
DVE 2x on HW (stays 1x) — plain tensor_tensor all-bf16 packed does;
GPSIMD tensor_tensor costs ~2.1us/[128,1024] (eff 0.42) and putting it
on the critical path drops engine p-states; stall-heavy schedules
inflate busy times ~20% (p-state), so the shallow PE->DVE->PE graph
with PSUM-accumulated merge beats deeper "balanced" graphs.
"""

import sys
import numpy as np

for _p in ("/opt/trn_rl_repo",):
    if _p not in sys.path:
        sys.path.insert(0, _p)

B, C, Dg, Hg, Wg = 4, 12, 8, 16, 16
H, W = 1024, 1024
N_CORES = 8
ROWS_PER_CORE = H // 2          # 512
N_G32 = ROWS_PER_CORE // 32     # 16 groups of 32 rows
NQ = C // 4                     # 3 channel quads


def _hat_matrix(n_out: int, n_lat: int) -> np.ndarray:
    i = np.linspace(0.0, n_lat - 1.0, n_out, dtype=np.float32)
    lat = np.arange(n_lat, dtype=np.float32)[:, None]
    return np.maximum(0.0, 1.0 - np.abs(i[None, :] - lat)).astype(np.float32)


def _build_tables():
    ax = _hat_matrix(W, Wg)                      # [16, 1024]
    ay = _hat_matrix(H, Hg)                      # [16, 1024]
    # sel32[(dd,y), y'] = -(y==y') : reduces 4 depth blocks -> 32 rows.
    # NEGATED because v tiles carry -w8*P (w8_neg = min(u-1,0) = -relu(1-u)
    # is one fused DVE tensor_scalar op); the sign cancels in the matmul.
    sel32 = np.zeros((128, 32), np.float32)
    for dd in range(4):
        sel32[dd * 32:(dd + 1) * 32, :] = -np.eye(32, dtype=np.float32)
    bias_lo = np.repeat(-np.arange(0, 4, dtype=np.float32), 32)[:, None]
    bias_hi = np.repeat(-np.arange(4, 8, dtype=np.float32), 32)[:, None]

    ay_bd = {}
    for r0 in (0, ROWS_PER_CORE):
        both = []
        for off in (0, 4):
            blk = np.zeros((N_G32, 128, 128), np.float32)
            a = ay[:, r0:r0 + ROWS_PER_CORE].reshape(16, N_G32, 32).transpose(1, 0, 2)
            for dd in range(4):
                d = dd + off
                blk[:, d * 16:(d + 1) * 16, dd * 32:(dd + 1) * 32] = a
            both.append(blk.transpose(1, 0, 2).reshape(128, N_G32 * 128))
        ay_bd[r0] = np.ascontiguousarray(
            np.stack(both, 1).reshape(128, 2 * N_G32 * 128))
    return dict(ax=ax, sel32=sel32, bias_lo=bias_lo, bias_hi=bias_hi, ay_bd=ay_bd)


def _build_nc():
    from contextlib import ExitStack
    import concourse.bass as bass
    import concourse.bacc as bacc
    import concourse.tile as tile
    import concourse.mybir as mybir

    f32 = mybir.dt.float32
    f32r = mybir.dt.float32r
    bf16 = mybir.dt.bfloat16
    AF = mybir.ActivationFunctionType
    Alu = mybir.AluOpType

    nc = bacc.Bacc("TRN2", target_bir_lowering=False, debug=False)

    guide_d = nc.dram_tensor("guide", [ROWS_PER_CORE, W], f32, kind="ExternalInput")
    gT_d = nc.dram_tensor("gT", [16, C * 128], f32, kind="ExternalInput")
    ax_d = nc.dram_tensor("ax", [16, W], f32, kind="ExternalInput")
    aybd_d = nc.dram_tensor("aybd", [128, 2 * N_G32 * 128], bf16, kind="ExternalInput")
    sel_d = nc.dram_tensor("sel32", [128, 32], bf16, kind="ExternalInput")
    bias_lo_d = nc.dram_tensor("bias_lo", [128, 1], f32, kind="ExternalInput")
    bias_hi_d = nc.dram_tensor("bias_hi", [128, 1], f32, kind="ExternalInput")
    # out[q, g, (cc,y32), (h,x)] bf16
    out_d = nc.dram_tensor("out", [NQ, N_G32, 128, 2 * 512], bf16,
                           kind="ExternalOutput")

    with tile.TileContext(nc) as tc, ExitStack() as ctx:
        const = ctx.enter_context(tc.tile_pool(name="const", bufs=1))
        ax_t = const.tile([16, W], f32)
        nc.sync.dma_start(ax_t[:], ax_d[:])
        gT_t = const.tile([16, C * 128], f32)
        nc.sync.dma_start(gT_t[:], gT_d[:])
        ay_r = const.tile([128, 2 * N_G32 * 128], bf16)
        nc.sync.dma_start(ay_r[:], aybd_d[:])
        sel_b = const.tile([128, 32], bf16)
        nc.sync.dma_start(sel_b[:], sel_d[:])
        bias_lo_t = const.tile([128, 1], f32)
        nc.sync.dma_start(bias_lo_t[:], bias_lo_d[:])
        bias_hi_t = const.tile([128, 1], f32)
        nc.sync.dma_start(bias_hi_t[:], bias_hi_d[:])
        # PE warmup: dummy matmuls on a zeroed tile so the HAM activity
        # window sees a busy PE and ramps 1.2->2.4 GHz before real work.
        # Emitted FIRST so the DVE queue runs them before the const copies
        # (which block on their DMAs).
        wm_t = const.tile([16, 512], f32)
        nc.vector.memset(wm_t[:], 0.0)
        wm_r = const.tile([16, 512], f32r)
        nc.vector.tensor_copy(wm_r[:], wm_t[:])

        gT_r = const.tile([16, C * 128], f32r)
        nc.vector.tensor_copy(gT_r[:], gT_t[:])
        ax_r = const.tile([16, W], f32r)
        nc.vector.tensor_copy(ax_r[:], ax_t[:])

        s_pool = ctx.enter_context(tc.tile_pool(name="s_all", bufs=1))
        s_tiles = []
        for c in range(C):
            s_c = s_pool.tile([128, W], bf16, tag=f"s{c}", name=f"s{c}")
            s_tiles.append(s_c)

        iz_pool = ctx.enter_context(tc.tile_pool(name="iz128", bufs=3))
        u8_pool = ctx.enter_context(tc.tile_pool(name="u8", bufs=4))
        w8_pool = ctx.enter_context(tc.tile_pool(name="w8", bufs=4))
        phs_pool = ctx.enter_context(tc.tile_pool(name="phs", bufs=6))
        vlo_pool = ctx.enter_context(tc.tile_pool(name="vlo", bufs=6))
        vhi_pool = ctx.enter_context(tc.tile_pool(name="vhi", bufs=6))
        # unified p8 PSUM pool: 3 x [128,1024] f32 = 6 banks; each (g,c)
        # draws hi then lo, so the pipeline is ~1.5 (g,c) deep uniformly.
        ps_p8 = ctx.enter_context(tc.tile_pool(name="ps_p8", bufs=3, space="PSUM"))
        ps_out = ctx.enter_context(tc.tile_pool(name="ps_out", bufs=1, space="PSUM"))
        ob_pool = ctx.enter_context(tc.tile_pool(name="ob", bufs=6))

        # Stage A (x-interp S[c] = gT[c].T @ Ax) is pipelined into the
        # g=0 channel loop with 2-channel lookahead; sa tiles borrow the
        # ps_p8 pool slots to stay within the 8 PSUM banks.
        def emit_stA(c):
            sa = ps_p8.tile([128, W], f32, tag="p8")
            for h in range(2):
                nc.tensor.matmul(
                    sa[:, h * 512:(h + 1) * 512],
                    gT_r[:, c * 128:(c + 1) * 128],
                    ax_r[:, h * 512:(h + 1) * 512],
                    start=True, stop=True)
            nc.scalar.copy(s_tiles[c][:], sa[:])

        for _ in range(6):
            wmp = ps_p8.tile([128, W], f32, tag="p8")
            for h in range(2):
                nc.tensor.matmul(wmp[:, h * 512:(h + 1) * 512],
                                 wm_r[:, :128], wm_r[:],
                                 start=True, stop=True)

        for c in range(2):
            emit_stA(c)

        # software-pipelined red: holds (v, c) pending (2-deep so slow
        # GPSIMD multiplies never stall the PE reduce matmuls)
        pending = []

        def emit_red(force=False):
            while pending and (force or len(pending) > 2):
                vs, c, otiles = pending.pop(0)
                cc = c % 4
                # h=1 first: its staging copy is on DVE (faster drain), so
                # the next quad's o1-writing matmuls unblock sooner.
                for h in (1, 0):
                    for i, v in enumerate(vs):
                        nc.tensor.matmul(
                            otiles[h][cc * 32:(cc + 1) * 32, :],
                            sel_b[:],
                            v[:, h * 512:(h + 1) * 512],
                            start=(i == 0), stop=(i == len(vs) - 1),
                            tile_position=(0, cc * 32),
                        )
                if cc == 3:
                    g = otiles[2]
                    q = c // 4
                    for h in (1, 0):
                        ob = ob_pool.tile([128, 512], bf16, tag=f"ob{h}")
                        if h == 0:
                            nc.scalar.copy(ob[:], otiles[h][:])
                        else:
                            nc.vector.tensor_copy(ob[:], otiles[h][:])
                        nc.sync.dma_start(out_d[q, g, :, h * 512:(h + 1) * 512],
                                          ob[:])

        # w8(g) is computed on ACT one group AHEAD (during g-1's slack
        # window at the copy-free direct channel) so DVE never stalls on
        # the abs/relu chain at a group boundary; guide DMA prefetched.
        def load_iz(g):
            iz = iz_pool.tile([128, W], f32)
            for r in range(4):
                nc.sync.dma_start(iz[r * 32:(r + 1) * 32, :],
                                  guide_d[bass.ts(g, 32), :])
            return iz

        def w8_act(iz):
            # u = |7*guide - d| on ACT; w8_neg = min(u-1, 0) = -relu(1-u)
            # as ONE DVE tensor_scalar (bf16 SBUF -> 4x mode, ~330ns).
            u_lo = u8_pool.tile([128, W], bf16, tag="u8_0")
            u_hi = u8_pool.tile([128, W], bf16, tag="u8_1")
            w8_lo = w8_pool.tile([128, W], bf16, tag="w8_0")
            w8_hi = w8_pool.tile([128, W], bf16, tag="w8_1")
            nc.scalar.activation(u_lo[:], iz[:], AF.Abs,
                                 bias=bias_lo_t[:], scale=7.0)
            nc.scalar.activation(u_hi[:], iz[:], AF.Abs,
                                 bias=bias_hi_t[:], scale=7.0)
            nc.vector.tensor_scalar(w8_lo[:], u_lo[:], 1.0, 0.0,
                                    op0=Alu.subtract, op1=Alu.min)
            nc.vector.tensor_scalar(w8_hi[:], u_hi[:], 1.0, 0.0,
                                    op0=Alu.subtract, op1=Alu.min)
            return w8_lo, w8_hi

        w8_cur = w8_act(load_iz(0))
        iz_next = None
        for g in range(N_G32):
            w8_lo, w8_hi = w8_cur
            if g + 1 < N_G32:
                iz_next = load_iz(g + 1)

            otiles = None
            for c in range(C):
                if g == 0 and c + 2 < C:
                    emit_stA(c + 2)
                if c == 5 and g + 1 < N_G32:
                    w8_cur = w8_act(iz_next)
                cc = c % 4
                if cc == 0:
                    o0 = ps_out.tile([128, 512], f32, tag="o0")
                    o1 = ps_out.tile([128, 512], f32, tag="o1")
                    otiles = (o0, o1, g)
                # mm2: y-interp, both stacks
                p8_hi = ps_p8.tile([128, W], f32, tag="p8")
                p8_lo = ps_p8.tile([128, W], f32, tag="p8")
                for lh, p8 in ((1, p8_hi), (0, p8_lo)):
                    lhs_off = (lh * N_G32 + g) * 128
                    for h in range(2):
                        nc.tensor.matmul(
                            p8[:, h * 512:(h + 1) * 512],
                            ay_r[:, lhs_off:lhs_off + 128],
                            s_tiles[c][:, h * 512:(h + 1) * 512],
                            start=True, stop=True)
                # three mult paths per tile, rotated by channel so DVE /
                # ACT / GPSIMD all stay below the PE pacer:
                #   D: DVE tensor_tensor straight from PSUM (1x, ~1.19us)
                #   X: ACT copy PSUM->SBUF bf16 + DVE all-bf16 TT (2x)
                #   G: ACT copy + GPSIMD TT (~2.4us, off critical path)
                lo_path = ("X" if c in (0, 3, 6, 9) else
                           "G" if c in (2, 8) else "D")
                hi_path = ("G" if c in (0, 3, 6, 10) else
                           "X" if c in (1, 5, 8) else "D")
                # ACT copies first (egress head start), then DVE directs,
                # then the dependent bf16 multiplies.
                srcs = {}
                for path, p8 in ((hi_path, p8_hi), (lo_path, p8_lo)):
                    if path != "D":
                        ph_s = phs_pool.tile([128, W], bf16)
                        nc.scalar.copy(ph_s[:], p8[:])
                        srcs[id(p8)] = ph_s
                v_lo = vlo_pool.tile([128, W], bf16)
                v_hi = vhi_pool.tile([128, W], bf16)
                for path, p8, v, w8 in ((lo_path, p8_lo, v_lo, w8_lo),
                                        (hi_path, p8_hi, v_hi, w8_hi)):
                    if path == "D":
                        nc.vector.tensor_mul(v[:], w8[:], p8[:])
                for path, p8, v, w8 in ((hi_path, p8_hi, v_hi, w8_hi),
                                        (lo_path, p8_lo, v_lo, w8_lo)):
                    if path == "X":
                        nc.vector.tensor_tensor(v[:], srcs[id(p8)][:],
                                                w8[:], op=Alu.mult)
                    elif path == "G":
                        nc.gpsimd.tensor_tensor(v[:], srcs[id(p8)][:],
                                                w8[:], op=Alu.mult)
                pending.append(((v_lo, v_hi), c, otiles))
                emit_red()
        emit_red(force=True)

    nc.compile()
    return nc


_NC = None


def _get_nc():
    global _NC
    if _NC is None:
        _NC = _build_nc()
    return _NC


def make_in_maps(grid: np.ndarray, guide: np.ndarray):
    tabs = _build_tables()
    in_maps = []
    for core in range(N_CORES):
        b, half = core // 2, core % 2
        r0 = half * ROWS_PER_CORE
        gT = np.ascontiguousarray(
            grid[b].transpose(3, 0, 1, 2).reshape(16, C * 128))
        import ml_dtypes
        bf = ml_dtypes.bfloat16
        in_maps.append({
            "guide": np.ascontiguousarray(guide[b, 0, r0:r0 + ROWS_PER_CORE, :]),
            "gT": gT,
            "ax": tabs["ax"],
            "aybd": tabs["ay_bd"][r0].astype(bf),
            "sel32": tabs["sel32"].astype(bf),
            "bias_lo": tabs["bias_lo"],
            "bias_hi": tabs["bias_hi"],
        })
    return in_maps


def assemble(results) -> np.ndarray:
    out = np.empty((B, C, H, W), np.float32)
    for core in range(N_CORES):
        b, half = core // 2, core % 2
        r0 = half * ROWS_PER_CORE
        arr = np.asarray(results[core]["out"], dtype=np.float32)
        # arr [q, g, (cc,y32), (h,x)] -> [c, rows, W]
        arr = arr.reshape(NQ, N_G32, 4, 32, W)
        arr = arr.transpose(0, 2, 1, 3, 4).reshape(C, ROWS_PER_CORE, W)
        out[b, :, r0:r0 + ROWS_PER_CORE, :] = arr
    return out


def kernel(grid, guide, output_size):
    from concourse.bass_utils import run_bass_kernel_spmd

    grid = np.asarray(grid, dtype=np.float32)
    guide = np.asarray(guide, dtype=np.float32)
    assert grid.shape == (B, C, Dg, Hg, Wg), grid.shape
    assert guide.shape == (B, 1, H, W), guide.shape

    nc = _get_nc()
    in_maps = make_in_maps(grid, guide)
    res = run_bass_kernel_spmd(nc, in_maps, list(range(N_CORES)))
    return assemble(res.results)

